# revision 1
# baseline (speedup 1.0000x reference)
"""Boolean reservoir kernel for Trainium2 (8 NeuronCores, data-parallel over samples).

Perf notes (measured): the SWDGE indirect1d ucode costs ~1.06us per call
(consuming one offset per partition) plus ~310ns dispatch, so the 64
gathers/step dominate.  This version adds two-sub-batch pipelining (SB=2,
so Pool never idles waiting on the DVE/PE chain) and spreads the gathers
over 4 SWDGE queues, which removes nearly all inter-instruction stall.

Reference computation (per sample m):
    res[input_nodes] = x[t]                      (scatter input bits)
    idx[n] = sum_k res[k] * primes[k] * W[n,k]   (masked prime-weighted sum)
    res    = lut[n, idx[n]]                      (per-node LUT bit lookup)
    ... 512 sequential steps, then readout = res @ readout_w.T + b

Kernel formulation:
  - Node positions are permuted so the 32 input nodes sit at positions 224..255
    (= SBUF partitions 112..127 with the n' = 2p+h layout).  PW rows for input
    nodes are zeroed; their (known-in-advance) contribution to idx is
    precomputed on the host per (t, m, n) and added on-device ("xc").
    Gathers for input-node positions are skipped except at the final step.
  - The LUT is bit-packed on the host into int32 words (8MB), so the per-(m,n)
    lookup is: word = lutbits[n'*8192 + (idx>>5)], bit = (word >> (idx&31)) & 1.
    The word fetch uses SWDGE indirect DMA.  The HW DynamicAP consumes ONE
    offset per partition per instruction (verified by probing), so each step
    issues 64 column gathers ([128,1] offsets -> 128 single-word descriptors
    each).  State matmul runs on PE in fp32 (exact for these integer ranges);
    index/offset/bit arithmetic runs on DVE int paths (verified exact).
"""

import sys

for _p in ("/opt/trn_rl_repo", "/root/.axon_site/_ro/trn_rl_repo"):
    if _p not in sys.path:
        sys.path.insert(0, _p)

import numpy as np

import concourse.bacc as bacc
import concourse.bass as bass
import concourse.mybir as mybir
import concourse.tile as tile
from concourse.alu_op_type import AluOpType
from concourse.bass import IndirectOffsetOnAxis
from concourse.bass_utils import run_bass_kernel_spmd

# Problem dims (hardcoded per spec)
R = 256
M = 256
S = 512
NB = 32          # d*b input bits
LUT_LEN = 2 ** 18
OUT = 32
NCORES = 8
MLOC = M // NCORES          # 32 samples per core
SB = 2                      # sub-batches per core
JB = MLOC // SB             # samples per sub-batch
FB2 = 2 * JB                # free size of res/idx tiles (h*JB + j)
WORDS_PER_NODE = LUT_LEN // 32   # 8192
NWORDS = R * WORDS_PER_NODE      # 2097152 (8MB int32)
NIN = 32                    # input nodes
P_FULL = 128
P_GATH = (R - NIN) // 2     # 112: partitions holding non-input nodes
XC_CHUNK = 8                # steps of xc prefetched per DMA

F32 = mybir.dt.float32
I32 = mybir.dt.int32


def build_bass(steps: int = S):
    """Build + compile the (input-independent) bass program."""
    nc = bacc.Bacc(
        "TRN2",
        target_bir_lowering=False,
        debug=False,
        enable_asserts=False,
        num_devices=NCORES,
        num_swdge_queues=4,
    )

    FB = FB2

    # DRAM tensors (names = in_map keys)
    lutbits_d = nc.dram_tensor("lutbits", [NWORDS, 1], I32, kind="ExternalInput")
    pw_d = nc.dram_tensor("pw", [2, 2, P_FULL, P_FULL], F32, kind="ExternalInput")
    rwt_d = nc.dram_tensor("rwt", [2, P_FULL, OUT], F32, kind="ExternalInput")
    bias_d = nc.dram_tensor("bias", [JB, OUT], F32, kind="ExternalInput")
    basew_d = nc.dram_tensor("basew", [P_FULL, FB], I32, kind="ExternalInput")
    res0_d = nc.dram_tensor("res0", [P_FULL, SB * FB], F32, kind="ExternalInput")
    nxc = (steps + XC_CHUNK - 1) // XC_CHUNK
    xc_d = nc.dram_tensor("xc", [nxc, P_FULL, XC_CHUNK * SB * FB], F32,
                          kind="ExternalInput")
    out_d = nc.dram_tensor("out", [MLOC, OUT], F32, kind="ExternalOutput")

    lut_ap = lutbits_d.ap()

    with tile.TileContext(nc) as tc:
        with (
            tc.tile_pool(name="const", bufs=1) as cpool,
            tc.tile_pool(name="state", bufs=1) as spool,
            tc.tile_pool(name="work", bufs=3) as wpool,
            tc.tile_pool(name="xc", bufs=2) as xcpool,
            tc.tile_pool(name="psum", bufs=2, space="PSUM") as ppool,
        ):
            # --- constants to SBUF ---
            pw_t = [[cpool.tile([P_FULL, P_FULL], F32, tag=f"pw{ho}{hk}", name=f"pw{ho}{hk}")
                     for hk in range(2)] for ho in range(2)]
            for ho in range(2):
                for hk in range(2):
                    nc.sync.dma_start(out=pw_t[ho][hk][:], in_=pw_d.ap()[ho, hk])
            rwt_t = [cpool.tile([P_FULL, OUT], F32, tag=f"rwt{h}", name=f"rwt{h}") for h in range(2)]
            for h in range(2):
                nc.sync.dma_start(out=rwt_t[h][:], in_=rwt_d.ap()[h])
            bias_t = cpool.tile([JB, OUT], F32, tag="bias", name="biast")
            nc.sync.dma_start(out=bias_t[:], in_=bias_d.ap())
            basew_t = cpool.tile([P_FULL, FB], I32, tag="basew", name="basewt")
            nc.sync.dma_start(out=basew_t[:], in_=basew_d.ap())
            five_t = cpool.tile([P_FULL, 1], I32, tag="five", name="fivet")
            nc.vector.memset(five_t[:], 5)

            # --- persistent state tiles per sub-batch ---
            res_t = [spool.tile([P_FULL, FB], F32, tag=f"res{sb}", name=f"res{sb}") for sb in range(SB)]
            words_t = [spool.tile([P_FULL, FB], I32, tag=f"words{sb}", name=f"words{sb}")
                       for sb in range(SB)]
            for sb in range(SB):
                nc.sync.dma_start(
                    out=res_t[sb][:],
                    in_=res0_d.ap()[:, sb * FB:(sb + 1) * FB],
                )
                # gathers skip partitions >= P_GATH except at the last step;
                # keep those rows defined so the bit-extract reads are clean
                nc.vector.memset(words_t[sb][:], 0)

            xc_tiles = {}

            for t in range(steps):
                ct, ti = divmod(t, XC_CHUNK)
                if ti == 0:
                    xct = xcpool.tile([P_FULL, XC_CHUNK * SB * FB], F32, tag="xc", name="xct")
                    nc.sync.dma_start(out=xct[:], in_=xc_d.ap()[ct])
                    xc_tiles = {"tile": xct}
                xct = xc_tiles["tile"]

                for sb in range(SB):
                    res = res_t[sb]
                    words = words_t[sb]

                    # state matmul: psum[p, ho*JB+j] = idx of node 2p+ho, sample j
                    psum = ppool.tile([P_FULL, FB], F32, space="PSUM", tag="psum", name="psumt")
                    for ho in range(2):
                        for hk in range(2):
                            nc.tensor.matmul(
                                psum[:, ho * JB:(ho + 1) * JB],
                                pw_t[ho][hk][:],
                                res[:, hk * JB:(hk + 1) * JB],
                                start=(hk == 0),
                                stop=(hk == 1),
                            )

                    xc_sl = xct[:, (ti * SB + sb) * FB:(ti * SB + sb + 1) * FB]

                    # idx = int32(psum + xc)   (exact: values < 2^18)
                    idx = wpool.tile([P_FULL, FB], I32, tag="idx", name="idxt")
                    nc.vector.scalar_tensor_tensor(
                        out=idx[:], in0=psum[:], scalar=1.0, in1=xc_sl,
                        op0=AluOpType.mult, op1=AluOpType.add,
                    )
                    # offs = (idx >> 5) | node_base_words  (base = n'*2^13,
                    # idx>>5 < 2^13, so OR == add and both ops are bitwise)
                    offs = wpool.tile([P_FULL, FB], I32, tag="offs", name="offst")
                    nc.vector.scalar_tensor_tensor(
                        out=offs[:], in0=idx[:], scalar=five_t[:, :1], in1=basew_t[:],
                        op0=AluOpType.logical_shift_right, op1=AluOpType.bitwise_or,
                    )
                    # bit = idx & 31
                    bit = wpool.tile([P_FULL, FB], I32, tag="bit", name="bitt")
                    nc.vector.tensor_scalar(
                        out=bit[:], in0=idx[:], scalar1=31, scalar2=None,
                        op0=AluOpType.bitwise_and,
                    )

                    # gather LUT words: one row-gather per column (HW
                    # consumes one offset per partition per call).  For
                    # t < last, skip partitions >= P_GATH (input-node rows:
                    # their PW contraction rows are zero, values unused).
                    pg = P_FULL if t == steps - 1 else P_GATH
                    for col in range(FB):
                        gi = nc.gpsimd.indirect_dma_start(
                            out=words[:pg, col:col + 1],
                            out_offset=None,
                            in_=lut_ap,
                            in_offset=IndirectOffsetOnAxis(
                                ap=offs[:pg, col:col + 1], axis=0),
                        )
                        q = col // 16
                        if q:
                            gi.ins.queue = "qPoolDynamic%d" % q

                    # res = float32((words >> bit) & 1)
                    sh = wpool.tile([P_FULL, FB], I32, tag="sh", name="sht")
                    nc.vector.tensor_tensor(
                        out=sh[:], in0=words[:], in1=bit[:],
                        op=AluOpType.logical_shift_right,
                    )
                    bi = wpool.tile([P_FULL, FB], I32, tag="bi", name="bit2")
                    nc.vector.tensor_scalar(
                        out=bi[:], in0=sh[:], scalar1=1, scalar2=None,
                        op0=AluOpType.bitwise_and,
                    )
                    nc.vector.tensor_copy(out=res[:], in_=bi[:])

            # --- readout ---
            for sb in range(SB):
                pro = ppool.tile([JB, OUT], F32, space="PSUM", tag="pro", name="prot")
                for h in range(2):
                    nc.tensor.matmul(
                        pro[:],
                        res_t[sb][:, h * JB:(h + 1) * JB],
                        rwt_t[h][:],
                        start=(h == 0),
                        stop=(h == 1),
                    )
                ro = wpool.tile([JB, OUT], F32, tag="ro", name="rot")
                nc.vector.tensor_tensor(
                    out=ro[:], in0=pro[:], in1=bias_t[:], op=AluOpType.add,
                )
                nc.sync.dma_start(
                    out=out_d.ap()[sb * JB:(sb + 1) * JB, :], in_=ro[:],
                )

    nc.compile()
    return nc


def prep_inputs(x, lut, init_res, W, primes, input_nodes, readout_w, readout_b,
                steps: int = S):
    """Host-side prep: permutation, weight relayout, LUT bit-pack, xc precompute.

    Returns (shared_map, per_core_maps)."""
    x = np.asarray(x)
    lut = np.asarray(lut, dtype=np.int32)
    init_res = np.asarray(init_res)
    W = np.asarray(W)
    primes = np.asarray(primes, dtype=np.int64)
    input_nodes = np.asarray(input_nodes, dtype=np.int64)
    readout_w = np.asarray(readout_w, dtype=np.float32)
    readout_b = np.asarray(readout_b, dtype=np.float32)

    m, s, d, b = x.shape
    assert (m, s, d * b) == (M, S, NB) and steps <= S

    others = np.array(sorted(set(range(R)) - set(input_nodes.tolist())),
                      dtype=np.int64)
    nodes_at = np.concatenate([others, input_nodes])  # position -> orig node id
    assert nodes_at.shape == (R,)

    # PW with input-node rows (contraction side) zeroed, permuted, as lhsT chunks
    c = (W.astype(np.int64) * primes[None, :]).astype(np.float64)  # c[n, k]
    c[:, input_nodes] = 0.0
    cp = c[np.ix_(nodes_at, nodes_at)]  # c'[n', k']
    pw = np.zeros((2, 2, P_FULL, P_FULL), dtype=np.float32)
    for ho in range(2):
        for hk in range(2):
            # pw[ho, hk, p_k, p_out] = c'[2*p_out+ho, 2*p_k+hk]
            pw[ho, hk] = cp[ho::2, hk::2].T.astype(np.float32)

    # bit-packed LUT in permuted node order
    lb = np.packbits(lut[nodes_at].astype(np.uint8), axis=1, bitorder="little")
    lutbits = np.ascontiguousarray(lb).view(np.int32).reshape(NWORDS, 1)
    assert lutbits.shape == (NWORDS, 1)

    # per-position word base
    n_of_pf = (2 * np.arange(P_FULL)[:, None]
               + (np.arange(2 * JB)[None, :] // JB))  # [128, FB2] -> n' = 2p+h
    basew = (n_of_pf * WORDS_PER_NODE).astype(np.int32)

    # res0[p, sb*32 + h*16 + j] = init_res[nodes_at[2p+h]]
    r0 = init_res[nodes_at].astype(np.float32)  # [256] by position
    res0 = np.broadcast_to(
        r0.reshape(P_FULL, 2)[:, None, :, None], (P_FULL, SB, 2, JB)
    ).reshape(P_FULL, SB * 2 * JB).copy()

    # readout weights by position: rwt[h, p, o] = readout_w[o, nodes_at[2p+h]]
    rwp = readout_w[:, nodes_at]  # [OUT, 256]
    rwt = np.stack([rwp[:, h::2].T for h in range(2)]).astype(np.float32)
    bias = np.broadcast_to(readout_b[None, :], (JB, OUT)).astype(np.float32).copy()

    # xc[t, m, n] = sum_j x[m, t, j] * primes[input_nodes[j]] * W[n, input_nodes[j]]
    xt = x.reshape(M, S, NB).astype(np.float32)
    cin = (primes[input_nodes][:, None]
           * W[:, input_nodes].astype(np.int64).T).astype(np.float32)  # [NB, n]
    xc_full = xt.reshape(M * S, NB) @ cin  # [M*S, 256] exact in fp32 (< 2^24)
    xc_full = xc_full.reshape(M, S, R)[:, :, nodes_at]  # by position n'

    nxc = (steps + XC_CHUNK - 1) // XC_CHUNK
    tpad = nxc * XC_CHUNK
    per_core = []
    for core in range(NCORES):
        xcc = xc_full[core * MLOC:(core + 1) * MLOC, :steps]  # [32, steps, 256]
        if tpad != steps:
            xcc = np.concatenate(
                [xcc, np.zeros((MLOC, tpad - steps, R), xcc.dtype)], axis=1)
        # -> [ct, p, ti, sb, h, j]
        arr = xcc.reshape(SB, JB, nxc, XC_CHUNK, P_FULL, 2)
        arr = arr.transpose(2, 4, 3, 0, 5, 1)
        per_core.append({"xc": np.ascontiguousarray(
            arr.reshape(nxc, P_FULL, XC_CHUNK * SB * 2 * JB), dtype=np.float32)})

    shared = dict(lutbits=lutbits, pw=pw, rwt=rwt, bias=bias, basew=basew,
                  res0=res0)
    return shared, per_core


_NC_CACHE = {}


def _get_nc(steps=S):
    if steps not in _NC_CACHE:
        _NC_CACHE[steps] = build_bass(steps)
    return _NC_CACHE[steps]


def kernel(**inputs) -> np.ndarray:
    nc = _get_nc(S)
    shared, per_core = prep_inputs(**inputs)
    in_maps = [{**shared, **pc} for pc in per_core]
    res = run_bass_kernel_spmd(nc, in_maps, core_ids=list(range(NCORES)))
    out = np.concatenate([np.asarray(r["out"]) for r in res.results], axis=0)
    return out.astype(np.float32)

